# revision 10
# baseline (speedup 1.0000x reference)
"""Trainium2 Bass kernel for a 2-layer feed-forward LIF recurrence.

Reference semantics (per time step, two stacked LIF cells, f32):
    vd = v + 0.2*(i - v);  id = i + 0.4*(-i)
    z  = (vd > 1);         v' = (1 - z) * vd;   i' = id + inp
layer1 input = x_t, layer2 input = z1_t, output = z2_t.

Rescaled state: U = 5*v, I = i (raw). Then
    y = 0.8*U + I;  z = (y > 5);  U' = (1-z)*y;  I' = 0.6*I + inp
with NO prescaling of x anywhere.

Two fused custom-DVE ops carry the whole recurrence:
  A  (both layers, 512 wide):  U' = select(0.8*(U*(U > -1e38)) + I > 5,
                                           -FLT_MAX, ...)
     decay+add+threshold+reset in ONE DVE pass; a spike writes the
     -FLT_MAX *sentinel*; the (U > -1e38) factor lazily cleans it to 0.
  C2 (layer-2 current, 256):   I2' = 0.6*I2 + (U1' < -1e38)
     consumes layer-1 spikes straight from the sentinel - z1 is never
     materialized (it is needed nowhere else).
Voltage state lives in the rotating staging block (one [U1|U2] slot per
step, written once), so layer-2 spike extraction z2 = (U2' < -1e38) runs
in bulk per 8-step block on the otherwise-idle Activation engine
(Sign+Relu -> exact {0.0, 1.0}).

Layer-1 current I1' = 0.6*I1 + x_t runs on Pool as ts(mult) + tt(add)
(Pool has no scalar_tensor_tensor in silicon - codegen rejects it).

Engine busy per step (per core; per-layer tile [128 x 256]):
  DVE  A 594 + C2 327            = 921
  Pool H1 450 + tt 603           = 1053
  ACT  bulk z2 (amortized)       = 473   (+ out-DMA queue)
  DMA  in 364 + out 364          = 728

Sharding: data-parallel over batch. B=16 -> 2 batches per core across 8
NeuronCores; the T=256 scan runs on-chip with state resident in SBUF.
"""
import numpy as np

import concourse.bass as bass
import concourse.bacc as bacc
import concourse.tile as tile
from concourse import mybir
from concourse.bass_utils import run_bass_kernel_spmd
from concourse.dve_ops import (
    DveOp,
    OPS,
    CUSTOM_DVE_SPECS,
    _SUB_OPCODE_FOR_NAME,
    _CUSTOM_DVE_ROW_BASE,
)
from concourse.dve_spec import Spec, Src0, Src1, C0, C1, C2, MaxNeg, select, lower
from concourse.dve_uop import DveOpSpec

T, B, H, W = 256, 16, 128, 128
NCORES = 8
BPC = B // NCORES            # batches per core
P = 128                      # SBUF partitions
F = (BPC * H * W) // P       # 256 free elems per layer per step
TBLK = 8                     # time steps per staging block

F32 = mybir.dt.float32
OP = mybir.AluOpType
AF = mybir.ActivationFunctionType

DEC_V = float(np.float32(1.0) - np.float32(1e-3 * 200.0))  # 0.8
DEC_I = float(np.float32(1.0) - np.float32(1e-3 * 400.0))  # 0.6
VTH = 5.0                    # threshold in U = 5*v scale
SENT_THR = -1e38             # anything below this is a spike sentinel
FMIN = float(np.finfo(np.float32).min)


def _ref_lif(in0, in1, s0, s1, imm2):
    """CoreSim reference for LIF_FUSED_ANT: in0=U, in1=I, s0=decay,
    s1=threshold, imm2=sentinel-detect bound."""
    ind = (imm2 < in0).astype(np.float32)
    y = ((in0.astype(np.float32) * ind) * s0 + in1).astype(np.float32)
    return np.where(s1 < y, np.float32(FMIN), y).astype(np.float32)


def _ref_i2(in0, in1, s0, s1, imm2):
    """CoreSim reference for LIF_I2_ANT: in0=I2, in1=U1', s0=decay,
    s1=sentinel-detect bound."""
    z = (in1 < s1).astype(np.float32)
    return ((in0.astype(np.float32) * s0) + z).astype(np.float32)


def _register_op(name, body, ref):
    spec = Spec(body=body, reference=ref)
    shas = {}
    for ver in ("v3", "v4"):
        try:
            shas[ver] = DveOpSpec(
                name=name, opcode=1, uops=lower(spec, ver=ver), rd1_en=True
            ).sha(ver)
        except ValueError:
            pass
    op = DveOp(name, spec, subdim=False, uops_sha=shas)
    if op.name not in _SUB_OPCODE_FOR_NAME:
        OPS.append(op)
        CUSTOM_DVE_SPECS[op.name] = op.spec
        _SUB_OPCODE_FOR_NAME[op.name] = _CUSTOM_DVE_ROW_BASE + len(OPS) - 1
    return op


_ind = C2 < Src0
_y = (Src0 * _ind) * C0 + Src1
LIF = _register_op("LIF_FUSED_ANT", select(C1 < _y, MaxNeg, _y), _ref_lif)
I2OP = _register_op("LIF_I2_ANT", Src0 * C0 + (Src1 < C1), _ref_i2)


def build_nc():
    nc = bacc.Bacc("TRN2")
    x_d = nc.declare_dram_parameter("x", [T, P, F], F32, isOutput=False)
    o_d = nc.declare_dram_parameter("out", [T, P, F], F32, isOutput=True)

    with tile.TileContext(nc) as tc:
        with (
            tc.tile_pool(name="state", bufs=1) as sp,
            tc.tile_pool(name="io", bufs=3) as iop,
        ):
            IA = sp.tile([P, 2 * F], F32, tag="IA")    # [I1 | I2], parity 0
            IB = sp.tile([P, 2 * F], F32, tag="IB")    # [I1 | I2], parity 1
            H1 = sp.tile([P, F], F32, tag="H1")        # Pool tmp: 0.6*I1
            UBOOT = sp.tile([P, 2 * F], F32, tag="UBOOT")
            BIASN = sp.tile([P, 1], F32, tag="BIASN")  # Sign bias: -1e38
            nc.vector.memset(IA[:], 0.0)
            nc.vector.memset(IB[:], 0.0)
            nc.gpsimd.memset(UBOOT[:], 0.0)
            nc.gpsimd.memset(BIASN[:], -1e38)

            uprev = UBOOT[:]
            for t0 in range(0, T, TBLK):
                XB = iop.tile([P, TBLK * F], F32, tag="xb")      # x staging
                UB = iop.tile([P, TBLK * 2 * F], F32, tag="ub")  # [U1|U2]/step
                ZB = iop.tile([P, TBLK * F], F32, tag="zb")      # z2 out
                nc.sync.dma_start(
                    XB[:].rearrange("p (t f) -> p t f", t=TBLK),
                    x_d[t0 : t0 + TBLK].rearrange("t p f -> p t f"),
                )
                for k in range(TBLK):
                    t = t0 + k
                    Icur = (IA, IB)[t % 2]
                    Inxt = (IA, IB)[(t + 1) % 2]
                    xs = XB[:, bass.ts(k, F)]
                    uslot = UB[:, bass.ts(k, 2 * F)]   # [U1_t | U2_t]
                    # A (DVE): fused decay/add/threshold/reset, both layers
                    nc.vector._custom_dve(
                        LIF, out=uslot, in0=uprev, in1=Icur[:],
                        s0=DEC_V, s1=VTH, imm2=SENT_THR,
                    )
                    # C2 (DVE): I2' = 0.6*I2 + (U1' < -1e38)
                    nc.vector._custom_dve(
                        I2OP, out=Inxt[:, F:], in0=Icur[:, F:],
                        in1=uslot[:, :F], s0=DEC_I, s1=SENT_THR,
                    )
                    # C1 (Pool): I1' = 0.6*I1 + x_t  as ts(mult) + tt(add)
                    nc.gpsimd.tensor_scalar(H1[:], Icur[:, :F], DEC_I, None,
                                            OP.mult)
                    nc.gpsimd.tensor_tensor(Inxt[:, :F], H1[:], xs, OP.add)
                    uprev = uslot
                # D (ACT, bulk): z2 = relu(sign(-U2' - 1e38)) in {0.0, 1.0}
                u2view = UB[:].rearrange("p (t two f) -> p t two f",
                                         t=TBLK, two=2)[:, :, 1, :]
                nc.scalar.activation(
                    ZB[:].rearrange("p (t f) -> p t f", t=TBLK), u2view,
                    AF.Sign, bias=BIASN[:], scale=-1.0)
                nc.scalar.activation(ZB[:], ZB[:], AF.Relu)
                # out-DMA from the ACT queue: SP's sequencer is held for the
                # whole DMA in the cost model; two queues de-serialize in/out.
                nc.scalar.dma_start(
                    o_d[t0 : t0 + TBLK].rearrange("t p f -> p t f"),
                    ZB[:].rearrange("p (t f) -> p t f", t=TBLK),
                )
    nc.compile()
    return nc


_NC_CACHE = {}


def _get_nc():
    if "nc" not in _NC_CACHE:
        _NC_CACHE["nc"] = build_nc()
    return _NC_CACHE["nc"]


def _shard_inputs(x):
    shards = []
    for c in range(NCORES):
        xs = np.ascontiguousarray(x[:, c * BPC : (c + 1) * BPC]).reshape(T, P, F)
        shards.append({"x": xs})
    return shards


def _unshard(outs):
    parts = [o.reshape(T, BPC, H, W) for o in outs]
    return np.concatenate(parts, axis=1)


def kernel(x, _trace=False):
    x = np.asarray(x)
    assert x.shape == (T, B, H, W), x.shape
    nc = _get_nc()
    res = run_bass_kernel_spmd(nc, _shard_inputs(x), list(range(NCORES)),
                               trace=_trace)
    out = _unshard([np.asarray(r["out"]) for r in res.results])
    if _trace:
        return out.astype(np.float32), res
    return out.astype(np.float32)
